# revision 1
# baseline (speedup 1.0000x reference)
"""Distributed Trainium2 kernel for AlternateWeaveGather (segment_reduce).

Reference computation:
    h = x @ W.T + b                      # [N, 512] linear
    out = segment_mean(h, batch, 256)    # [256, 512]

Since the linear layer commutes with the segment sum:
    out[s] = (segsum_x[s] @ W.T) / max(c[s], 1) + b * (c[s] > 0)

so each core only needs to segment-reduce its row shard of x (one-hot
matmul on the TensorEngine), ReduceScatter the [256, 513] (sums|counts)
across the 8 cores, and run the tiny 512x512 linear on its 32 owned
segments. This turns a 68-GFLOP matmul problem into an HBM-bound
streaming reduction.

Because batch is sorted, each core's 16384 rows span only ~33 contiguous
segment ids, so the one-hot window is 128 wide (one matmul per 128 rows
instead of two 256-wide ones). The host passes batch ids relative to the
core's first segment; an indirect-scatter DMA realigns the local
[128, 513] window into global segment rows before the ReduceScatter.

Sharding: data-parallel over rows. x/batch split along dim 0 across 8
cores; W/b replicated; ReduceScatter combines local sums+counts before
the divide; host concatenates the 8x[32, 512] outputs.
"""

import numpy as np

import concourse.bacc as bacc
import concourse.bass as bass
import concourse.mybir as mybir
import concourse.tile as tile
from concourse.bass_utils import run_bass_kernel_spmd

N_CORES = 8
N_ROWS = 131072
D = 512
N_SEG = 256
SEG_PER_CORE = N_SEG // N_CORES
W_WIN = 128  # one-hot window width (per-core segment span is ~33)

F32 = mybir.dt.float32
F32R = mybir.dt.float32r
I32 = mybir.dt.int32
BF16 = mybir.dt.bfloat16
RS_BF16 = True  # ship the ReduceScatter payload in bf16 (halves transfer)


def build_nc(rows_per_core=N_ROWS // N_CORES):
    assert rows_per_core % 1024 == 0
    n_sup = rows_per_core // 1024  # supertiles of 1024 rows (16KB DMA lines)

    nc = bacc.Bacc("TRN2", target_bir_lowering=False, debug=False,
                   num_devices=N_CORES)
    x = nc.dram_tensor("x", [rows_per_core, D], F32, kind="ExternalInput")
    # batchp[p, k*n_sup + t] = batch_rel[1024t + 8p + k]  (8 row planes)
    batchp = nc.dram_tensor("batchp", [128, 8 * n_sup], F32,
                            kind="ExternalInput")
    idx = nc.dram_tensor("idx", [128, 1], I32, kind="ExternalInput")
    wt = nc.dram_tensor("wt", [D, D], BF16, kind="ExternalInput")
    bb = nc.dram_tensor("bb", [SEG_PER_CORE, D], F32, kind="ExternalInput")
    out = nc.dram_tensor("out", [SEG_PER_CORE, D], F32, kind="ExternalOutput")

    iota_c = nc.inline_tensor(
        np.tile(np.arange(W_WIN, dtype=np.float32), (128, 1)).astype(
            mybir.dt.np(BF16)), name="iota_c")
    ident_c = nc.inline_tensor(np.eye(128, dtype=np.float32), name="ident_c")
    ones_c = nc.inline_tensor(np.ones((128, 2), dtype=np.float32).astype(
        mybir.dt.np(BF16)), name="ones_c")
    zeros_c = nc.inline_tensor(np.zeros((129, D + 1), dtype=np.float32),
                               name="zeros_c")
    zeros_bf = nc.inline_tensor(
        np.zeros((129, D + 1), dtype=np.float32), name="zeros_bf")
    sel_c = nc.inline_tensor(
        np.tile(np.eye(SEG_PER_CORE, dtype=np.float32), (4, 1)).astype(
            mybir.dt.np(BF16)), name="sel_c")
    CCDT = BF16 if RS_BF16 else F32

    # [n_sup, 128, 8, 512]; per (t, p) the (8, 512) block is 16KB contiguous
    x_r = x.ap().rearrange("(t p k) d -> t p k d", p=128, k=8)
    RG = [list(range(N_CORES))]

    with tile.TileContext(nc) as tc:
        with tc.tile_pool(name="const", bufs=1) as const, \
             tc.tile_pool(name="dram", bufs=1, space="DRAM") as dram:
            iota_sb = const.tile([128, W_WIN], BF16, name="iota_sb")
            batch_sb = const.tile([128, 8 * n_sup], F32, name="batch_sb")
            idx_sb = const.tile([128, 1], I32, name="idx_sb")
            ones_sb = const.tile([128, 2], BF16, name="ones_sb")
            wt_sb = const.tile([128, 4 * D], BF16, name="wt_sb")
            b_sb = const.tile([SEG_PER_CORE, D], F32, name="b_sb")
            ohacc = const.tile([128, W_WIN], BF16, name="ohacc")
            nc.sync.dma_start(out=iota_sb[:, :], in_=iota_c[:, :])
            nc.sync.dma_start(out=batch_sb[:, :], in_=batchp[:, :])
            nc.gpsimd.dma_start(out=idx_sb[:, :], in_=idx[:, :])
            nc.gpsimd.dma_start(out=ones_sb[:, :], in_=ones_c[:, :])
            nc.gpsimd.dma_start(
                out=ohacc[:, :],
                in_=zeros_bf[0:128, 0:W_WIN // 2].bitcast(BF16))

            rs_in = dram.tile([N_SEG + 1, D + 1], CCDT, name="rs_in")
            rs_out = dram.tile([N_SEG, D + 1], CCDT, name="rs_out")
            zsrc = zeros_bf.ap().bitcast(CCDT) if RS_BF16 else zeros_c.ap()
            # zero the scatter target (only 128 of 257 rows get data)
            nc.gpsimd.dma_start(out=rs_in[0:129, :],
                                 in_=zsrc[0:129, 0:D + 1])
            nc.gpsimd.dma_start(out=rs_in[129:257, :],
                                in_=zsrc[0:128, 0:D + 1])

            # fire-and-forget tiny AllReduce: warms the ncfw collective
            # path (cold doorbell->poll costs ~12us) while the main loop
            # streams; nothing depends on its output
            warm_in = dram.tile([128, 2], F32, name="warm_in")
            warm_out = dram.tile([128, 2], F32, name="warm_out",
                                 addr_space="Shared")
            warm_dma = nc.gpsimd.dma_start(out=warm_in[:, :],
                                           in_=zeros_c[0:128, 0:2])
            warm_cc = nc.gpsimd.collective_compute(
                "AllReduce", mybir.AluOpType.add, replica_groups=RG,
                ins=[warm_in.opt()], outs=[warm_out.opt()])
            bass._add_dep_helper(warm_cc.ins, warm_dma.ins, False,
                                 "warm AR right after its input")

            with tc.tile_pool(name="xin", bufs=4) as xp, \
                 tc.tile_pool(name="ohp", bufs=12) as ohp, \
                 tc.tile_pool(name="psum_acc", bufs=1, space="PSUM") as pacc:
                ps = pacc.tile([128, D], F32, name="ps")
                for t in range(n_sup):
                    xt = xp.tile([128, 8, D], F32, name="xt")
                    xq = nc.sync if t % 2 == 0 else nc.scalar
                    if t == n_sup - 1:
                        # split the final supertile so the tail of the
                        # pipeline drains per-2-plane, not per-8-plane
                        for c in range(4):
                            q2 = nc.sync if c % 2 == 0 else nc.scalar
                            q2.dma_start(out=xt[:, 2 * c:2 * c + 2, :],
                                         in_=x_r[t][:, 2 * c:2 * c + 2, :])
                    else:
                        xq.dma_start(out=xt[:, :, :], in_=x_r[t])
                    # little-endian f32: the high halfword of each element
                    # IS its truncated bf16 value -> free bf16 operand
                    xt_bf = xt[:, :, :].bitcast(BF16)
                    st, sp0 = (t == 0), (t == n_sup - 1)
                    for k in range(8):
                        oh = ohp.tile([128, W_WIN], BF16, name="oh")
                        nc.vector.tensor_scalar(
                            oh[:, :], iota_sb[:, :],
                            batch_sb[:, k * n_sup + t:k * n_sup + t + 1],
                            None, mybir.AluOpType.is_equal)
                        nc.tensor.matmul(ps[:, :], oh[:, :],
                                         xt_bf[:, k, 1::2],
                                         start=(st and k == 0),
                                         stop=(sp0 and k == 7),
                                         skip_group_check=True)
                        # counts only need the first 64 window columns
                        # (per-core span is ~33; host asserts <= 64);
                        # same engine as is_eq so the accumulator never
                        # drains late
                        nc.vector.tensor_tensor(
                            ohacc[:, 0:64], ohacc[:, 0:64],
                            oh[:, 0:64], mybir.AluOpType.add)

                with tc.tile_pool(name="half", bufs=1) as halfp, \
                     tc.tile_pool(name="psum_cnt", bufs=1,
                                  space="PSUM") as pcnt:
                    # counts: column sums of the accumulated one-hots
                    pc = pcnt.tile([128, 2], F32, name="pc")
                    nc.tensor.matmul(pc[:, :], ohacc[:, :], ones_sb[:, :],
                                     start=True, stop=True)
                    sbw = halfp.tile([128, D + 1], CCDT, name="sbw")
                    nc.vector.tensor_copy(sbw[:, 0:D // 2], ps[:, 0:D // 2])
                    nc.scalar.copy(sbw[:, D // 2:D], ps[:, D // 2:D])
                    nc.scalar.copy(sbw[:, D:D + 1], pc[:, 0:1])
                    # place the local window at its global segment rows
                    # (row p -> rs_in[idx[p]]; idx clamps overflow to the
                    # dummy row 256)
                    nc.gpsimd.indirect_dma_start(
                        out=rs_in[:, :], out_offset=bass.IndirectOffsetOnAxis(
                            ap=idx_sb[:, 0:1], axis=0),
                        in_=sbw[:, :], in_offset=None)

            # AllToAll (pure copy, ~2x the wire rate of ReduceScatter):
            # block i of the output is rank i's partial for MY 32 segments;
            # the 8-way sum happens on-core right after
            nc.gpsimd.collective_compute(
                "AllToAll", mybir.AluOpType.bypass, replica_groups=RG,
                ins=[rs_in[0:N_SEG, :].opt()], outs=[rs_out.opt()])

            # epilogue inputs (only needed after the collective)
            for i in range(4):
                nc.scalar.dma_start(out=wt_sb[:, i * D:(i + 1) * D],
                                    in_=wt[i * 128:(i + 1) * 128, :])
            nc.scalar.dma_start(out=b_sb[:, :], in_=bb[:, :])

            with tc.tile_pool(name="epi", bufs=1) as epi, \
                 tc.tile_pool(name="psum_epi", bufs=1, space="PSUM") as pepi:
                sel_sb = epi.tile([128, SEG_PER_CORE], BF16, name="sel_sb")
                nc.scalar.dma_start(out=sel_sb[:, :], in_=sel_c[:, :])
                # my 32 segments' partials from all 8 ranks: rows (i*32+s)
                blk0 = epi.tile([128, D + 1], CCDT, name="blk0")
                blk1 = epi.tile([128, D + 1], CCDT, name="blk1")
                nc.sync.dma_start(out=blk0[:, :], in_=rs_out[0:128, :])
                nc.scalar.dma_start(out=blk1[:, :], in_=rs_out[128:256, :])

                # fused 8-block sum + transpose on the TensorEngine:
                # pt_c[d, s] = sum_p blk[p, d_c] * sel[p, s]
                lhsT = epi.tile([128, 4 * SEG_PER_CORE], BF16, name="lhsT")
                for c in range(4):
                    pt = pepi.tile([128, SEG_PER_CORE], F32, name="pt",
                                   tag="pt", bufs=2)
                    nc.tensor.matmul(pt[:, :],
                                     blk0[:, c * 128:(c + 1) * 128],
                                     sel_sb[:, :], start=True, stop=False)
                    nc.tensor.matmul(pt[:, :],
                                     blk1[:, c * 128:(c + 1) * 128],
                                     sel_sb[:, :], start=False, stop=True)
                    if c % 2 == 0:
                        nc.vector.tensor_copy(
                            lhsT[:, c * SEG_PER_CORE:(c + 1) * SEG_PER_CORE],
                            pt[:, :])
                    else:
                        nc.scalar.copy(
                            lhsT[:, c * SEG_PER_CORE:(c + 1) * SEG_PER_CORE],
                            pt[:, :])
                # counts: cnt[s] = sum_p sel[p, s] * blk[p, 512]
                pcnt2 = pepi.tile([SEG_PER_CORE, 1], F32, name="pcnt2")
                nc.tensor.matmul(pcnt2[:, :], sel_sb[:, :],
                                 blk0[:, D:D + 1], start=True, stop=False)
                nc.tensor.matmul(pcnt2[:, :], sel_sb[:, :],
                                 blk1[:, D:D + 1], start=False, stop=True)

                cm = epi.tile([SEG_PER_CORE, 1], F32, name="cm")
                inv = epi.tile([SEG_PER_CORE, 1], F32, name="inv")
                ind = epi.tile([SEG_PER_CORE, 1], F32, name="ind")
                bind = epi.tile([SEG_PER_CORE, D], F32, name="bind")
                nc.vector.tensor_scalar_max(cm[:, :], pcnt2[:, :], 1.0)
                nc.vector.reciprocal(inv[:, :], cm[:, :])
                nc.vector.tensor_scalar_min(ind[:, :], pcnt2[:, :], 1.0)
                nc.vector.tensor_scalar(bind[:, :], b_sb[:, :], ind[:, 0:1],
                                        None, mybir.AluOpType.mult)

                po = pepi.tile([SEG_PER_CORE, D], F32, name="po")
                for i in range(4):
                    nc.tensor.matmul(
                        po[:, :],
                        lhsT[:, i * SEG_PER_CORE:(i + 1) * SEG_PER_CORE],
                        wt_sb[:, i * D:(i + 1) * D],
                        start=(i == 0), stop=(i == 3))
                res = epi.tile([SEG_PER_CORE, D], F32, name="res")
                # res = (sums @ Wt) / max(c,1) + b*min(c,1)
                nc.vector.scalar_tensor_tensor(
                    res[:, :], po[:, :], inv[:, 0:1],
                    bind[:, :], mybir.AluOpType.mult, mybir.AluOpType.add)
                nc.sync.dma_start(out=out[:, :], in_=res[:, :])
    nc.compile()
    return nc


def make_in_maps(x, W, b, batch, rows_per_core):
    x = np.asarray(x, dtype=np.float32)
    W = np.asarray(W, dtype=np.float32)
    b = np.asarray(b, dtype=np.float32)
    batch = np.asarray(batch)
    n_sup = rows_per_core // 1024
    wt = np.ascontiguousarray(W.T).astype(mybir.dt.np(BF16))
    bb = np.ascontiguousarray(np.tile(b.reshape(1, D), (SEG_PER_CORE, 1)))
    in_maps = []
    for j in range(N_CORES):
        lo = j * rows_per_core
        bs = batch[lo:lo + rows_per_core].astype(np.int64)
        base = int(bs[0])
        rel = (bs - base).astype(np.float32)
        assert rel.max() < W_WIN, (
            f"core {j}: segment span {int(rel.max()) + 1} exceeds window")
        assert rel.max() < 64, f"core {j}: span {int(rel.max()) + 1} > 64"
        planes = rel.reshape(n_sup, 128, 8)
        bp = np.concatenate([planes[:, :, k].T for k in range(8)], axis=1)
        rowidx = np.minimum(base + np.arange(128), N_SEG).astype(np.int32)
        in_maps.append({
            "x": np.ascontiguousarray(x[lo:lo + rows_per_core]),
            "batchp": np.ascontiguousarray(bp),
            "idx": np.ascontiguousarray(rowidx.reshape(128, 1)),
            "wt": wt,
            "bb": bb,
        })
    return in_maps


_NC_CACHE = {}


def kernel(x, W, b, batch, num_segments, trace=False):
    assert int(num_segments) == N_SEG
    rows_per_core = N_ROWS // N_CORES
    if rows_per_core not in _NC_CACHE:
        _NC_CACHE[rows_per_core] = build_nc(rows_per_core)
    nc = _NC_CACHE[rows_per_core]
    in_maps = make_in_maps(x, W, b, batch, rows_per_core)
    res = run_bass_kernel_spmd(nc, in_maps, core_ids=list(range(N_CORES)),
                               trace=trace)
    full = np.concatenate([res.results[j]["out"] for j in range(N_CORES)],
                          axis=0)
    if trace:
        return full, res
    return full

